# revision 2
# baseline (speedup 1.0000x reference)
"""Attention-LSTM captioning model, data-parallel over batch on 8 NeuronCores.

Contract: kernel(**inputs) takes FULL (unsharded) numpy inputs keyed as in
setup_inputs() and returns the FULL [B, T-1, V] float32 output.

Sharding: batch 64 -> 8 cores x 8 rows (data-parallel, weights replicated).
The embedding gather (emb[seq]) is host-side indexing; everything else runs
on the NeuronCores via a single jitted shard_map program.

HBM-traffic optimizations vs the naive unrolled loop (which streamed ~45MB of
fp32 weights per step = ~730MB total, ~2ms at 358GB/s/core):
  * all large operands stored bf16 in HBM (host-side cast) -> half traffic
  * token part of the gate GEMM hoisted out of the loop: gates_x is computed
    for all 16 steps in one GEMM, so Wih[:, :H] streams once, not 16x
  * output head batched: logits/log_softmax for all 16 steps in one GEMM
    after the recurrence, so logit_W (19MB fp32) streams once, not 16x
Per-step residual traffic: Wih[:, H:] (8.4MB bf16) + Whh (2MB) + h2a (0.5MB)
+ att_feats (6.4MB) + p_att (1.6MB) ~= 19MB/step -> ~300MB total vs ~730MB.
"""

import numpy as np
import jax
import jax.numpy as jnp
from jax.sharding import Mesh, PartitionSpec as P
from jax.experimental.shard_map import shard_map

N_CORES = 8
H = 512
F = 2048
V = 9488
L = 196
T = 17  # seq length; T-1 = 16 recurrent steps

_CACHE = {}

BF = jnp.bfloat16
F32 = jnp.float32


def _mm(a, b):
    # bf16 operands, fp32 accumulate.
    return jnp.matmul(a, b, preferred_element_type=F32)


def _model(fc, att, xts, lin_W, lin_b, Wih_x, Wih_a, Whh, ctx_W, ctx_b,
           h2a_W, h2a_b, alpha_W, alpha_b, logit_W, logit_b):
    # Per-core shapes: fc [b,F] f32, att [b,L,F] bf16, xts [b,T-1,H] bf16.
    # Weights: lin_W f32; Wih_x [4H,H], Wih_a [4H,F], Whh, ctx_W, h2a_W,
    # logit_W bf16; biases f32.
    b = fc.shape[0]
    h = fc @ lin_W.T + lin_b                       # [b,H] f32
    c = h
    # ctx2att, constant across steps. bf16 GEMM, f32 accum, then bf16 store.
    p_att = (jnp.einsum('blf,hf->blh', att, ctx_W,
                        preferred_element_type=F32) + ctx_b).astype(BF)
    # Token part of the LSTM gate GEMM for all steps at once: Wih_x reads once.
    gates_x = jnp.einsum('bth,gh->btg', xts, Wih_x,
                         preferred_element_type=F32)   # [b,T-1,4H] f32
    hs = []
    for t in range(T - 1):
        att_h = _mm(h.astype(BF), h2a_W.T) + h2a_b            # [b,H] f32
        dot = jnp.tanh(p_att + att_h[:, None, :].astype(BF))  # [b,L,H] bf16
        e = jnp.einsum('blh,h->bl', dot, alpha_W[0],
                       preferred_element_type=F32) + alpha_b[0]
        w = jax.nn.softmax(e, axis=-1)                        # [b,L] f32
        att_res = jnp.einsum('bl,blf->bf', w.astype(BF), att,
                             preferred_element_type=F32)      # [b,F] f32
        gates = (gates_x[:, t, :]
                 + _mm(att_res.astype(BF), Wih_a.T)
                 + _mm(h.astype(BF), Whh.T))                  # [b,4H] f32
        i_g = gates[:, 0 * H:1 * H]
        f_g = gates[:, 1 * H:2 * H]
        g_g = gates[:, 2 * H:3 * H]
        o_g = gates[:, 3 * H:4 * H]
        c = jax.nn.sigmoid(f_g) * c + jax.nn.sigmoid(i_g) * jnp.tanh(g_g)
        h = jax.nn.sigmoid(o_g) * jnp.tanh(c)
        hs.append(h)
    # Output head batched over time: logit_W streams once; [b*(T-1), H] rows
    # give the PE a full-height stationary tile instead of 8-row slivers.
    h_all = jnp.stack(hs, axis=1).reshape(b * (T - 1), H)     # [b*16, H] f32
    logits = _mm(h_all.astype(BF), logit_W.T) + logit_b       # [b*16, V] f32
    logp = jax.nn.log_softmax(logits, axis=-1)
    return logp.reshape(b, T - 1, V)


# (name, sharded_over_batch?) in _model argument order.
ARG_SPEC = [
    ('fc', True), ('att', True), ('xts', True),
    ('lin_W', False), ('lin_b', False),
    ('Wih_x', False), ('Wih_a', False), ('Whh', False),
    ('ctx_W', False), ('ctx_b', False),
    ('h2a_W', False), ('h2a_b', False),
    ('alpha_W', False), ('alpha_b', False),
    ('logit_W', False), ('logit_b', False),
]


def get_compiled():
    """Jitted SPMD function over the 8 NeuronCores (cached)."""
    if 'fn' in _CACHE:
        return _CACHE['fn'], _CACHE['mesh']
    devs = jax.devices()[:N_CORES]
    assert len(devs) == N_CORES, f"need {N_CORES} devices, have {jax.devices()}"
    mesh = Mesh(np.asarray(devs), ('core',))
    in_specs = tuple(P('core') if s else P() for _, s in ARG_SPEC)
    fn = jax.jit(shard_map(
        _model, mesh=mesh,
        in_specs=in_specs,
        out_specs=P('core'),
        check_rep=False,
    ))
    _CACHE['fn'] = fn
    _CACHE['mesh'] = mesh
    return fn, mesh


def prepare_args(fc_feats, att_feats, seq, lin_W, lin_b, emb, Wih, Whh,
                 ctx_W, ctx_b, h2a_W, h2a_b, alpha_W, alpha_b,
                 logit_W, logit_b):
    """Host-side preprocessing: embedding gather + dtype normalization.

    Large tensors are cast to bf16 on host so HBM holds (and the kernel
    streams) half-width data.
    """
    import ml_dtypes
    bf16 = ml_dtypes.bfloat16
    f32 = np.float32
    seq = np.asarray(seq)
    emb_np = np.asarray(emb, f32)
    xts = emb_np[seq[:, :-1]]                      # [B,T-1,H] host gather
    Wih = np.asarray(Wih, f32)
    args = (
        np.asarray(fc_feats, f32),
        np.asarray(att_feats, f32).astype(bf16),
        np.ascontiguousarray(xts).astype(bf16),
        np.asarray(lin_W, f32), np.asarray(lin_b, f32),
        np.ascontiguousarray(Wih[:, :H]).astype(bf16),
        np.ascontiguousarray(Wih[:, H:]).astype(bf16),
        np.asarray(Whh, f32).astype(bf16),
        np.asarray(ctx_W, f32).astype(bf16), np.asarray(ctx_b, f32),
        np.asarray(h2a_W, f32).astype(bf16), np.asarray(h2a_b, f32),
        np.asarray(alpha_W, f32).astype(bf16), np.asarray(alpha_b, f32),
        np.asarray(logit_W, f32).astype(bf16), np.asarray(logit_b, f32),
    )
    return args


def kernel(fc_feats, att_feats, seq, lin_W, lin_b, emb, Wih, Whh,
           ctx_W, ctx_b, h2a_W, h2a_b, alpha_W, alpha_b,
           logit_W, logit_b):
    args = prepare_args(fc_feats, att_feats, seq, lin_W, lin_b, emb, Wih, Whh,
                        ctx_W, ctx_b, h2a_W, h2a_b, alpha_W, alpha_b,
                        logit_W, logit_b)
    fn, _ = get_compiled()
    out = fn(*args)
    return np.asarray(jax.block_until_ready(out), np.float32)


# revision 3
# speedup vs baseline: 4.9342x; 4.9342x over previous
"""Attention-LSTM captioning model, data-parallel over batch on 8 NeuronCores.

Contract: kernel(**inputs) takes FULL (unsharded) numpy inputs keyed as in
setup_inputs() and returns the FULL [B, T-1, V] float32 output.

Sharding: batch 64 -> 8 cores x 8 rows (data-parallel, weights replicated).
The embedding gather (emb[seq]) is host-side indexing; everything else runs
on the NeuronCores via a single jitted shard_map program.

HBM-traffic optimizations vs the naive unrolled loop (which streamed ~45MB of
fp32 weights per step = ~730MB total, ~2ms at 358GB/s/core):
  * all large operands stored bf16 in HBM (host-side cast) -> half traffic
  * token part of the gate GEMM hoisted out of the loop: gates_x is computed
    for all 16 steps in one GEMM, so Wih[:, :H] streams once, not 16x
  * output head batched: logits/log_softmax for all 16 steps in one GEMM
    after the recurrence, so logit_W (19MB fp32) streams once, not 16x
Per-step residual traffic: Wih[:, H:] (8.4MB bf16) + Whh (2MB) + h2a (0.5MB)
+ att_feats (6.4MB) + p_att (1.6MB) ~= 19MB/step -> ~300MB total vs ~730MB.
"""

import numpy as np
import jax
import jax.numpy as jnp
from jax.sharding import Mesh, PartitionSpec as P
from jax.experimental.shard_map import shard_map

N_CORES = 8
H = 512
F = 2048
V = 9488
L = 196
T = 17  # seq length; T-1 = 16 recurrent steps

_CACHE = {}

BF = jnp.bfloat16
F32 = jnp.float32


def _mm(a, b):
    # bf16 operands, fp32 accumulate.
    return jnp.matmul(a, b, preferred_element_type=F32)


def _model(fc, att, xts, lin_W, lin_b, Wih_x, Wih_a, Whh, ctx_W, ctx_b,
           h2a_W, h2a_b, alpha_W, alpha_b, logit_W, logit_b):
    # Per-core shapes: fc [b,F] f32, att [b,L,F] bf16, xts [b,T-1,H] bf16.
    # Weights: lin_W f32; Wih_x [4H,H], Wih_a [4H,F], Whh, ctx_W, h2a_W,
    # logit_W bf16; biases f32.
    b = fc.shape[0]
    h = fc @ lin_W.T + lin_b                       # [b,H] f32
    c = h
    # ctx2att, constant across steps. bf16 GEMM, f32 accum, then bf16 store.
    p_att = (jnp.einsum('blf,hf->blh', att, ctx_W,
                        preferred_element_type=F32) + ctx_b).astype(BF)
    # Token part of the LSTM gate GEMM for all steps at once: Wih_x reads once.
    gates_x = jnp.einsum('bth,gh->btg', xts, Wih_x,
                         preferred_element_type=F32)   # [b,T-1,4H] f32
    att2d = att.reshape(b * L, F)                  # [b*L,F] bf16 (free view)
    alpha_col = alpha_W[0][:, None]                # [H,1] bf16
    eye_b = jnp.eye(b, dtype=BF)
    hs = []
    for t in range(T - 1):
        att_h = _mm(h.astype(BF), h2a_W.T) + h2a_b            # [b,H] f32
        dot = jnp.tanh(p_att + att_h[:, None, :].astype(BF))  # [b,L,H] bf16
        # e as one tall GEMM [b*L,H]@[H,1] instead of a batched matvec.
        e = _mm(dot.reshape(b * L, H), alpha_col).reshape(b, L) + alpha_b[0]
        w = jax.nn.softmax(e, axis=-1)                        # [b,L] f32
        # att_res as ONE [b, b*L]@[b*L, F] GEMM via block-diagonal weights
        # (the PE streams att2d once either way; avoids 8 M=1 batched GEMMs).
        w_bd = (w.astype(BF)[None, :, :] * eye_b[:, :, None]).reshape(b, b * L)
        att_res = _mm(w_bd, att2d)                            # [b,F] f32
        gates = (gates_x[:, t, :]
                 + _mm(att_res.astype(BF), Wih_a.T)
                 + _mm(h.astype(BF), Whh.T))                  # [b,4H] f32
        i_g = gates[:, 0 * H:1 * H]
        f_g = gates[:, 1 * H:2 * H]
        g_g = gates[:, 2 * H:3 * H]
        o_g = gates[:, 3 * H:4 * H]
        c = jax.nn.sigmoid(f_g) * c + jax.nn.sigmoid(i_g) * jnp.tanh(g_g)
        h = jax.nn.sigmoid(o_g) * jnp.tanh(c)
        hs.append(h)
    # Output head batched over time: logit_W streams once; [b*(T-1), H] rows
    # give the PE a full-height stationary tile instead of 8-row slivers.
    h_all = jnp.stack(hs, axis=1).reshape(b * (T - 1), H)     # [b*16, H] f32
    logits = _mm(h_all.astype(BF), logit_W.T) + logit_b       # [b*16, V] f32
    logp = jax.nn.log_softmax(logits, axis=-1)
    return logp.reshape(b, T - 1, V)


# (name, sharded_over_batch?) in _model argument order.
ARG_SPEC = [
    ('fc', True), ('att', True), ('xts', True),
    ('lin_W', False), ('lin_b', False),
    ('Wih_x', False), ('Wih_a', False), ('Whh', False),
    ('ctx_W', False), ('ctx_b', False),
    ('h2a_W', False), ('h2a_b', False),
    ('alpha_W', False), ('alpha_b', False),
    ('logit_W', False), ('logit_b', False),
]


def get_compiled():
    """Jitted SPMD function over the 8 NeuronCores (cached)."""
    if 'fn' in _CACHE:
        return _CACHE['fn'], _CACHE['mesh']
    devs = jax.devices()[:N_CORES]
    assert len(devs) == N_CORES, f"need {N_CORES} devices, have {jax.devices()}"
    mesh = Mesh(np.asarray(devs), ('core',))
    in_specs = tuple(P('core') if s else P() for _, s in ARG_SPEC)
    fn = jax.jit(shard_map(
        _model, mesh=mesh,
        in_specs=in_specs,
        out_specs=P('core'),
        check_rep=False,
    ))
    _CACHE['fn'] = fn
    _CACHE['mesh'] = mesh
    return fn, mesh


def prepare_args(fc_feats, att_feats, seq, lin_W, lin_b, emb, Wih, Whh,
                 ctx_W, ctx_b, h2a_W, h2a_b, alpha_W, alpha_b,
                 logit_W, logit_b):
    """Host-side preprocessing: embedding gather + dtype normalization.

    Large tensors are cast to bf16 on host so HBM holds (and the kernel
    streams) half-width data.
    """
    import ml_dtypes
    bf16 = ml_dtypes.bfloat16
    f32 = np.float32
    seq = np.asarray(seq)
    emb_np = np.asarray(emb, f32)
    xts = emb_np[seq[:, :-1]]                      # [B,T-1,H] host gather
    Wih = np.asarray(Wih, f32)
    args = (
        np.asarray(fc_feats, f32),
        np.asarray(att_feats, f32).astype(bf16),
        np.ascontiguousarray(xts).astype(bf16),
        np.asarray(lin_W, f32), np.asarray(lin_b, f32),
        np.ascontiguousarray(Wih[:, :H]).astype(bf16),
        np.ascontiguousarray(Wih[:, H:]).astype(bf16),
        np.asarray(Whh, f32).astype(bf16),
        np.asarray(ctx_W, f32).astype(bf16), np.asarray(ctx_b, f32),
        np.asarray(h2a_W, f32).astype(bf16), np.asarray(h2a_b, f32),
        np.asarray(alpha_W, f32).astype(bf16), np.asarray(alpha_b, f32),
        np.asarray(logit_W, f32).astype(bf16), np.asarray(logit_b, f32),
    )
    return args


def kernel(fc_feats, att_feats, seq, lin_W, lin_b, emb, Wih, Whh,
           ctx_W, ctx_b, h2a_W, h2a_b, alpha_W, alpha_b,
           logit_W, logit_b):
    args = prepare_args(fc_feats, att_feats, seq, lin_W, lin_b, emb, Wih, Whh,
                        ctx_W, ctx_b, h2a_W, h2a_b, alpha_W, alpha_b,
                        logit_W, logit_b)
    fn, _ = get_compiled()
    out = fn(*args)
    return np.asarray(jax.block_until_ready(out), np.float32)
